# revision 28
# baseline (speedup 1.0000x reference)
"""Trainium2 Bass kernel for nn_CausalSelfAttention_17368847745133.

Sharding (8 NeuronCores): core (b, g) = batch b in 0..3 x head-group g in
0..1 (8 heads each; Megatron column/row-parallel c_attn / c_proj).  The host
passes x[b].T so every device matmul runs transpose-free:

  qT/kT [512,2048] : matmul(lhsT=W_q|k slice, rhs=xT)      (transposed proj)
  V     [2048,512] : matmul(lhsT=xT tile, rhs=W_v slice)   (natural layout)
  S^T   [k,q]      : matmul(lhsT=kT head, rhs=qT head)     (d=64 contraction,
                     head pairs packed on PE row-groups 0-63 / 64-127; the two
                     matmuls stream concurrently through disjoint PE rows)
  P^T   = exp((S^T) / 8) * causal01                masked lanes exactly 0
  U     [65,q]     : matmul(lhsT=[V_head|ones], rhs=P^T)   row 64 = denom
  y^T   = U[0:64] * bcast(qm / denom)
  oT    [1024,2048]: matmul(lhsT=W_proj rows, rhs=y^T); host sums the two
                     group partials, transposes, adds b_proj, and overwrites
                     rows q >= l[b] with the (identical) uniform-softmax row.

All compute windows are trimmed to the causal triangle at 128-column
granularity (PSUM sub-range accumulation instead of zero-fill memsets) and to
q < Q_STOP = max(l).  Rows q in [l_b, Q_STOP) get qm = 0 so their device
output is zeroed before the host overwrite; the host recomputes those rows
exactly (softmax over an all-masked row is uniform, so y = mean_t v).
All matmuls run in bf16; softmax statistics stay fp32.
"""

import math

import ml_dtypes
import numpy as np

import concourse.bass as bass
import concourse.mybir as mybir
import concourse.tile as tile
from concourse import bacc
from concourse.bass_utils import run_bass_kernel_spmd

P = 128
B, T, C = 4, 2048, 1024
H, D = 16, 64
G = 2
HPG = H // G     # 8 heads per core
CG = HPG * D     # 512 channels per group
F32 = mybir.dt.float32
BF16 = mybir.dt.bfloat16
SCALE = 0.125    # 1/sqrt(64)

_CACHED = {}
_END = object()


def build_nc(q_stop):
    """Compile the SPMD program.  q_stop = max over batches of l[b]; rows
    q >= q_stop are never computed (host fills them)."""
    nc = bacc.Bacc(trn_type="TRN2", target_bir_lowering=False)

    xT = nc.dram_tensor("xT", [C, T], BF16, kind="ExternalInput")
    wq = nc.dram_tensor("wq", [P, 8, CG], BF16, kind="ExternalInput")
    wk = nc.dram_tensor("wk", [P, 8, CG], BF16, kind="ExternalInput")
    wv = nc.dram_tensor("wv", [P, 8, CG], BF16, kind="ExternalInput")
    wp = nc.dram_tensor("wp", [P, 4, C], BF16, kind="ExternalInput")
    qmA = nc.dram_tensor("qmA", [2, 4, 512], F32, kind="ExternalInput")
    m01 = nc.dram_tensor("m01", [P, P], BF16, kind="ExternalInput")
    oT = nc.dram_tensor("oT", [C, T], F32, kind="ExternalOutput")

    # per-(j-block) q width and kt count
    WJ = [min(512, q_stop - 512 * j) for j in range(4)]
    NKT = [min(4 * (j + 1), math.ceil((512 * j + max(WJ[j], 0)) / P))
           for j in range(4)]

    with tile.TileContext(nc) as tc:
        with tc.tile_pool(name="xtp", bufs=8) as xtp, \
             tc.tile_pool(name="qk", bufs=1) as qkpool, \
             tc.tile_pool(name="vp", bufs=1) as vpool, \
             tc.tile_pool(name="w", bufs=4) as wpool, \
             tc.tile_pool(name="pt", bufs=6) as ptpool, \
             tc.tile_pool(name="misc", bufs=1) as misc, \
             tc.tile_pool(name="norm", bufs=2) as norm, \
             tc.tile_pool(name="ob", bufs=3) as obpool, \
             tc.tile_pool(name="rdram", bufs=1, space="DRAM") as rdram, \
             tc.tile_pool(name="psS", bufs=2, space="PSUM") as psS, \
             tc.tile_pool(name="psQ", bufs=1, space="PSUM") as psQ, \
             tc.tile_pool(name="psU", bufs=2, space="PSUM") as psU:

            # ---- inputs ----
            wv_sb = wpool.tile([P, 8, CG], BF16, tag="w", name="wv_sb")
            nc.sync.dma_start(wv_sb, wv[:])
            xt = []
            for ct in range(8):
                t = xtp.tile([P, T], BF16, tag="xt", name=f"xt{ct}")
                nc.sync.dma_start(t, xT[ct * P:(ct + 1) * P, :])
                xt.append(t)
            wq_sb = wpool.tile([P, 8, CG], BF16, tag="w", name="wq_sb")
            nc.sync.dma_start(wq_sb, wq[:])
            wk_sb = wpool.tile([P, 8, CG], BF16, tag="w", name="wk_sb")
            nc.sync.dma_start(wk_sb, wk[:])
            m01_sb = misc.tile([P, P], BF16, tag="m01")
            nc.sync.dma_start(m01_sb, m01[:])
            qmA_sb = misc.tile([2, 4, 512], F32, tag="qmA")
            nc.sync.dma_start(qmA_sb, qmA[:])
            wp_sb = wpool.tile([P, 4, C], BF16, tag="w", name="wp_sb")
            nc.sync.dma_start(wp_sb, wp[:])

            dend = rdram.tile([32, 512], F32, tag="dend")
            qT_sb = qkpool.tile([P, 4, T], BF16, tag="qT")
            kT_sb = qkpool.tile([P, 4, T], BF16, tag="kT")
            V_sb = vpool.tile([P, 16, HPG, D + 1], BF16, tag="V")
            yT_sb = vpool.tile([P, 4, T], BF16, tag="yT")

            nc.vector.memset(V_sb[:, :, :, D:D + 1], 1.0)

            # ---- Phase A: V projection, kt-major in two 8-bank waves so
            # matmuls start as soon as xt[0] lands ----
            def v_wave(tts, wv_idx):
                pa = psS.tile([P, 2, 512], F32, tag="psS", name=f"vw{wv_idx}a")
                pb = psS.tile([P, 2, 512], F32, tag="psS", name=f"vw{wv_idx}b")
                pc = psQ.tile([P, 2, 512], F32, tag="psQ", name=f"vw{wv_idx}c")
                pd = psU.tile([P, 512], F32, tag="psU", name=f"vw{wv_idx}d")
                pe = psU.tile([P, 512], F32, tag="psU", name=f"vw{wv_idx}e")
                slots = [pa[:, 0], pa[:, 1], pb[:, 0], pb[:, 1],
                         pc[:, 0], pc[:, 1], pd, pe]
                for kt in range(8):
                    for i, tt in enumerate(tts):
                        nc.tensor.matmul(
                            slots[i], xt[kt][:, tt * P:(tt + 1) * P],
                            wv_sb[:, kt, :], start=(kt == 0), stop=(kt == 7))
                for i, tt in enumerate(tts):
                    nc.vector.tensor_copy(
                        V_sb[:, tt, :, 0:D],
                        slots[i].rearrange("p (h d) -> p h d", h=HPG))

            v_wave(range(0, 8), 0)
            v_wave(range(8, 16), 1)

            # ---- q/k projection chains.  Each (side, nbh) block is one
            # 16-matmul accumulation into the dedicated psQ bank pair; for
            # hp >= 1 the chain is interleaved into the previous head pair's
            # attention kt-steps to keep the PE array hot while Scalar paces
            # the softmax. ----
            def qk_chain(side, hp, nbh, pool=None, tag="psQ"):
                w_sb = wq_sb if side == 0 else wk_sb
                dst = qT_sb if side == 0 else kT_sb
                pool = psQ if pool is None else pool
                pt_ = pool.tile([P, 2, 512], F32, tag=tag,
                                name=f"qk{side}_{hp}_{nbh}")
                for kt in range(8):
                    for nb2 in range(2):
                        t0 = nbh * 1024 + nb2 * 512
                        w = 512
                        if side == 0:
                            w = min(512, max(q_stop - t0, 0))
                        if w > 0:
                            nc.tensor.matmul(
                                pt_[:, nb2, 0:w],
                                w_sb[:, kt, hp * P:(hp + 1) * P],
                                xt[kt][:, t0:t0 + w],
                                start=(kt == 0), stop=(kt == 7))
                        yield
                nc.vector.tensor_copy(
                    dst[:, hp, nbh * 1024:(nbh + 1) * 1024],
                    pt_.rearrange("p a b -> p (a b)"))

            def drain(gen):
                if gen is not None:
                    for _ in gen:
                        pass

            # chain order per hp matches when the next hp's attention needs
            # each block: (k,0) and (q,0) first
            CHAIN_ARGS = [(1, 0), (0, 0), (1, 1), (0, 1)]
            # filler assignment per j-block, sized to each block's step count
            CHAINS_FOR_J = [[], [(1, 0)], [(1, 1)], [(0, 0), (0, 1)]]

            # ---- Phase B: attention, one head pair per 128-row PE group ----
            for ci, (side, nbh) in enumerate(CHAIN_ARGS):
                if ci % 2 == 0:
                    drain(qk_chain(side, 0, nbh, pool=psS, tag="psS"))
                else:
                    drain(qk_chain(side, 0, nbh))
            for hp in range(4):
                chains_started = set()
                for j in range(4):
                    W = WJ[j]
                    if W <= 0:
                        continue
                    nkt = NKT[j]
                    chain = None
                    nmm = 0
                    if hp < 3:
                        specs = CHAINS_FOR_J[j]
                        chains_started.update(specs)
                        gens = [qk_chain(s, hp + 1, n) for s, n in specs]
                        nmm = 16 * len(gens)

                        def _cat(gens=gens):
                            for g in gens:
                                yield from g
                        chain = _cat() if gens else None
                    per_step = max(-(-nmm // nkt), 1)
                    Upr = [psU.tile([D + 1, 512], F32, tag="psU",
                                    name=f"U_{hp}_{j}_{par}")
                           for par in range(2)]

                    def s_exp(kt, j=j, hp=hp, W=W):
                        c0 = max(P * kt - 512 * j, 0)
                        if hp == 3 and kt % 3 == 2:
                            ss = psQ.tile([P, 2, 512], F32, tag="psQ",
                                          name="ssq")
                        else:
                            ss = psS.tile([P, 2, 512], F32, tag="psS")
                        for par in range(2):
                            p0 = par * D
                            nc.tensor.matmul(
                                ss[:, par, c0:W],
                                kT_sb[p0:p0 + D, hp, kt * P:(kt + 1) * P],
                                qT_sb[p0:p0 + D, hp,
                                      512 * j + c0:512 * j + W],
                                start=True, stop=True)
                        pt = ptpool.tile([P, 2, 512], BF16, tag="pt")
                        nc.scalar.activation(
                            pt[:, :, c0:W], ss[:, :, c0:W],
                            mybir.ActivationFunctionType.Exp,
                            bias=0.0, scale=SCALE)
                        if P * kt - 512 * j >= 0:
                            ce = min(c0 + P, W)
                            nc.vector.tensor_mul(
                                out=pt[:, :, c0:ce],
                                in0=pt[:, :, c0:ce],
                                in1=m01_sb[:, None, 0:ce - c0].to_broadcast(
                                    [P, 2, ce - c0]))
                        return pt, c0

                    def pv(kt, pt_c0, nkt=nkt, hp=hp, W=W, Upr=Upr):
                        pt, c0 = pt_c0
                        for par in range(2):
                            h = 2 * hp + par
                            nc.tensor.matmul(
                                Upr[par][:, c0:W],
                                V_sb[:, kt, h, :],
                                pt[:, par, c0:W],
                                start=(kt == 0), stop=(kt == nkt - 1))

                    if hp < 3:
                        prev = None
                        for kt in range(nkt):
                            cur = s_exp(kt)
                            if prev is not None:
                                pv(kt - 1, prev)
                            prev = cur
                            if chain is not None:
                                for _ in range(per_step):
                                    if next(chain, _END) is _END:
                                        chain = None
                                        break
                        pv(nkt - 1, prev)
                        drain(chain)
                    else:
                        # no fillers left for the last head pair: run the
                        # softmax two steps behind S so PV never waits on
                        # exp latency (third ss slot borrowed from psQ)
                        saved = []
                        for kt in range(nkt):
                            saved.append(s_exp(kt))
                            if kt >= 2:
                                pv(kt - 2, saved[kt - 2])
                        for kt in range(max(nkt - 2, 0), nkt):
                            pv(kt, saved[kt])

                    # epilogue: denominators, normalization scale, y^T.
                    # den lives on partition 64 of each U; stage it there
                    # (DVE is lane-local) and let DMA do the partition move.
                    r0 = 8 * hp + 2 * j
                    dtf = norm.tile([D + 1, 2, 512], F32, tag="dtf")
                    nc.vector.tensor_copy(dtf[D:D + 1, 0, 0:W],
                                          Upr[0][D:D + 1, 0:W])
                    nc.vector.tensor_copy(dtf[D:D + 1, 1, 0:W],
                                          Upr[1][D:D + 1, 0:W])
                    dent = norm.tile([2, 512], F32, tag="dent")
                    nc.sync.dma_start(dent[:, 0:W], dtf[D:D + 1, :, 0:W])
                    dqt = norm.tile([2, 512], F32, tag="dqt")
                    nc.vector.reciprocal_approx_fast(
                        dqt[:, 0:W], dent[:, 0:W])
                    nc.vector.tensor_mul(out=dqt[:, 0:W], in0=dqt[:, 0:W],
                                         in1=qmA_sb[:, j, 0:W])
                    nc.sync.dma_start(dend[r0:r0 + 2, 0:W], dqt[:, 0:W])
                    blk = slice(512 * j, 512 * j + W)
                    rb = norm.tile([P, 512], F32, tag="rb")
                    for par in range(2):
                        row = dend[r0 + par:r0 + par + 1, 0:W]
                        src = bass.AP(
                            tensor=row.tensor, offset=row.offset,
                            ap=[[0, D]] + list(row.ap[1:]))
                        nc.sync.dma_start(rb[par * D:(par + 1) * D, 0:W], src)
                    nc.vector.tensor_copy(yT_sb[0:D, hp, blk],
                                          Upr[0][0:D, 0:W])
                    ytmp = norm.tile([D, 512], BF16, tag="ytmp")
                    nc.vector.tensor_copy(ytmp[:, 0:W], Upr[1][0:D, 0:W])
                    nc.sync.dma_start(yT_sb[D:P, hp, blk], ytmp[:, 0:W])
                    ys = yT_sb[:, hp, blk]
                    nc.vector.tensor_mul(out=ys, in0=ys, in1=rb[:, 0:W])

                if hp < 3:
                    for spec in CHAIN_ARGS:
                        if spec not in chains_started:
                            drain(qk_chain(spec[0], hp + 1, spec[1]))

            # ---- Phase C: output projection.  qb blocks 0-2 first for
            # every output tile; the qb=3 columns depend on the very last
            # attention epilogue, so they run as a second pass after ~20us of
            # pass-1 matmuls have covered that latency. ----
            for mt in range(8):
                psa = psS.tile([P, 2, 512], F32, tag="psS", name=f"po{mt}a")
                psb = psQ.tile([P, 2, 512], F32, tag="psQ", name=f"po{mt}b")
                for ct in range(4):
                    for qb in range(3):
                        W = WJ[qb]
                        if W <= 0:
                            continue
                        out = psa[:, qb] if qb < 2 else psb[:, 0]
                        nc.tensor.matmul(
                            out[:, 0:W],
                            wp_sb[:, ct, mt * P:(mt + 1) * P],
                            yT_sb[:, ct, qb * 512:qb * 512 + W],
                            start=(ct == 0), stop=(ct == 3))
                ot = obpool.tile([P, 1024], F32, tag="ob")
                nc.scalar.copy(ot, psa.rearrange("p a b -> p (a b)"))
                nc.sync.dma_start(oT[mt * P:(mt + 1) * P, 0:1024], ot)
                ot2 = obpool.tile([P, 512], F32, tag="ob2")
                nc.vector.tensor_copy(ot2, psb[:, 0])
                nc.sync.dma_start(oT[mt * P:(mt + 1) * P, 1024:1536], ot2)
            if WJ[3] > 0:
                W = WJ[3]
                for mt in range(8):
                    ps3 = psU.tile([P, 512], F32, tag="psU", name=f"po{mt}c")
                    for ct in range(4):
                        nc.tensor.matmul(
                            ps3[:, 0:W],
                            wp_sb[:, ct, mt * P:(mt + 1) * P],
                            yT_sb[:, ct, 1536:1536 + W],
                            start=(ct == 0), stop=(ct == 3))
                    ot3 = obpool.tile([P, 512], F32, tag="ob2")
                    if mt % 2 == 0:
                        nc.scalar.copy(ot3[:, 0:W], ps3[:, 0:W])
                    else:
                        nc.vector.tensor_copy(ot3[:, 0:W], ps3[:, 0:W])
                    nc.sync.dma_start(
                        oT[mt * P:(mt + 1) * P, 1536:1536 + W], ot3[:, 0:W])

    nc.compile()
    return nc


def _bf(a):
    return np.ascontiguousarray(np.asarray(a)).astype(ml_dtypes.bfloat16)


def _prep_inputs(x, l, W_attn, W_proj, q_stop):
    x = np.asarray(x, dtype=np.float32)
    W_attn = np.asarray(W_attn, dtype=np.float32)
    W_proj = np.asarray(W_proj, dtype=np.float32)
    lv = np.asarray(l).astype(np.int64)

    m01 = np.where(np.arange(P)[:, None] > np.arange(P)[None, :],
                   0.0, 1.0).astype(ml_dtypes.bfloat16)

    in_maps = []
    for b in range(B):
        xTb = np.ascontiguousarray(x[b].T).astype(ml_dtypes.bfloat16)
        lb = int(np.clip(lv[b], 0, T))
        qrow = (np.arange(T) < lb).astype(np.float32)
        qmA = np.zeros((2, 4, 512), dtype=np.float32)
        for j in range(4):
            qmA[:, j] = qrow[512 * j:512 * (j + 1)][None, :]
        for g in range(2):
            cs = slice(g * CG, (g + 1) * CG)
            wqg = _bf(
                W_attn[:, 0:C][:, cs].reshape(8, P, CG).transpose(1, 0, 2))
            wkg = _bf(
                W_attn[:, C:2 * C][:, cs].reshape(8, P, CG).transpose(1, 0, 2))
            wvg = _bf(
                W_attn[:, 2 * C:3 * C][:, cs].reshape(8, P, CG).transpose(1, 0, 2))
            wpg = _bf(
                W_proj[cs, :].reshape(4, P, C).transpose(1, 0, 2))
            in_maps.append({
                "xT": xTb, "wq": wqg, "wk": wkg, "wv": wvg, "wp": wpg,
                "qmA": qmA, "m01": m01,
            })
    return in_maps


def kernel(x, l, W_attn, b_attn, W_proj, b_proj, _want_profile=False):
    x = np.asarray(x, dtype=np.float32)
    W_attn = np.asarray(W_attn, dtype=np.float32)
    W_proj = np.asarray(W_proj, dtype=np.float32)
    b_attn = np.asarray(b_attn, dtype=np.float32)
    b_proj = np.asarray(b_proj, dtype=np.float32)
    assert not np.any(b_attn), "nonzero b_attn not supported by this kernel"
    lv = np.asarray(l).astype(np.int64)
    q_stop = int(np.clip(lv.max(), 1, T))

    if q_stop not in _CACHED:
        _CACHED[q_stop] = build_nc(q_stop)
    nc = _CACHED[q_stop]

    in_maps = _prep_inputs(x, lv, W_attn, W_proj, q_stop)
    res = run_bass_kernel_spmd(nc, in_maps, core_ids=list(range(8)),
                               trace=_want_profile)

    # pad rows q >= l[b]: softmax over an all-masked row is uniform, so
    # y = mean_t v = (mean_t x[b]) @ W_v -> one output row per batch
    Wv_full = W_attn[:, 2 * C:3 * C].astype(np.float64)
    Wp64 = W_proj.astype(np.float64)
    out = np.empty((B, T, C), dtype=np.float32)
    for b in range(B):
        acc = res.results[2 * b]["oT"] + res.results[2 * b + 1]["oT"]
        out[b] = acc.T + b_proj[None, :]
        lb = int(np.clip(lv[b], 0, T))
        if lb < T:
            ypad = x[b].astype(np.float64).mean(axis=0) @ Wv_full
            padrow = (ypad @ Wp64 + b_proj.astype(np.float64)).astype(
                np.float32)
            out[b, lb:] = padrow[None, :]
    if _want_profile:
        return out, res
    return out
